# revision 15
# baseline (speedup 1.0000x reference)
"""Trainium2 Bass kernel for nn_Attention (B=4, L=1024, D=768, H=12, DH=64).

FP8 DoubleRow version, host-precomputed A.  Math per (batch b, head n):
    A = q_n^T k_n                 [D, D]  (weights-only -> precomputed host-side)
    scores = x A x^T              [L, L]
    S = softmax(scores, -1);  out = S V^T  with V = v_n x^T

Precision scheme (validated in numpy sim):
    host:  q,k scaled by 1024 -> fp8;  A_ps = q8^T k8 (f32, = A_true*2^20,
           entries ~341 +- 11);  A_f8 = fp8((A_ps - 341.333)*2)  <- mean-
           subtracted residual, shipped to the device directly.
           x -> fp8 (and bf16 copy for the V path);  v stays bf16.
           r = rowsums of x (bf16):  shipped as rcol [128, LB] (partition
           layout) and crb [128, L] (= 21.333*r broadcast along partitions).
    WT_f8 = fp8(A_f8^T x8 * 2^-5)           <- residual only
    sT_ps = x8 @ WT_f8;  then one fused DVE op per half adds the exact
           rank-1 mean term:  p = (crb * rcol) + p
    pt = exp(sT_ps * 2^-16) bf16
    R^T accumulated per l-block over m-blocks with vt_aug (V^T | ones),
    normalized by the ones-column sums.

Sharding: 48 (b,n) units over 8 cores as 4 batch-pairs x 4 head-triples.
No collectives: A comes from the host per head-triple.
"""

from contextlib import ExitStack

import ml_dtypes
import numpy as np

import concourse.tile as tile
from concourse import bacc, mybir
from concourse.bass import ts, ds
from concourse.bass_utils import run_bass_kernel_spmd

# If BASS_TRACE is set in an environment that lacks antenv.axon_hooks,
# run_bass_kernel_spmd's trace path would fail on import; register a
# fallback holder (and re-create the NTFF hook boot() skips when the
# module is missing at sitecustomize time) so tracing works/degrades
# gracefully instead.
try:
    import antenv.axon_hooks  # noqa: F401
except Exception:  # pragma: no cover
    import sys
    import types

    import antenv

    _m = types.ModuleType("antenv.axon_hooks")
    _m._hook = None
    _m.set_axon_ntff_profile_hook = lambda h: setattr(_m, "_hook", h)
    _m.get_axon_ntff_profile_hook = lambda: _m._hook
    sys.modules["antenv.axon_hooks"] = _m
    antenv.axon_hooks = _m
    try:
        from trn_agent_boot.trn_boot import _ntff_profile_via_ctypes

        _hook = _ntff_profile_via_ctypes("/opt/axon/libaxon_pjrt.so")
        if _hook is not None:
            _m.set_axon_ntff_profile_hook(_hook)
    except Exception:
        pass

B, L, D, H = 4, 1024, 768, 12
DH = D // H          # 64
HPC = 3              # heads per core
BPC = 2              # batches per core
N_CORES = 8
DC = D // 128        # 6 chunks of the contraction/feature dim
LB = L // 128        # 8 l-blocks / m-blocks
DHA = DH + 1         # 65: head slice width in vt_aug (ones column at 64)
F32 = mybir.dt.float32
BF16 = mybir.dt.bfloat16
F8 = mybir.dt.float8e4
DR = mybir.MatmulPerfMode.DoubleRow

C_PS = 0.25 * (2.0 ** 20) / D      # 341.333: expected A_ps mean
SA2 = 2.0                          # A residual scale
SW = 2.0 ** -5                     # WT residual copy scale
CR_SCALE = C_PS * SA2 * SW         # 21.333: correction row scale
EXP_SCALE = 1.0 / ((2.0 ** 20) * SA2 * SW)   # 2^-16

_COMPILED = None


def _build():
    nc = bacc.Bacc(
        "TRN2",
        target_bir_lowering=False,
        debug=False,
        enable_asserts=False,
        num_devices=N_CORES,
    )
    xTb_ext = nc.dram_tensor("xTb", [BPC, D, L], BF16, kind="ExternalInput").ap()
    xT8_ext = nc.dram_tensor("xT8", [BPC, D, L], F8, kind="ExternalInput").ap()
    a3_ext = nc.dram_tensor("a3", [HPC, D, D], F8, kind="ExternalInput").ap()
    vT3_ext = nc.dram_tensor("vT3", [D, HPC * DH], BF16, kind="ExternalInput").ap()
    crb_ext = nc.dram_tensor("crb", [BPC, 128, L], BF16, kind="ExternalInput").ap()
    rcol_ext = nc.dram_tensor("rcol", [BPC, 128, LB], BF16, kind="ExternalInput").ap()
    # raw numerators + ones-column denominators; the softmax divide
    # happens on the host (removes ~100 DVE ops from the drain windows)
    out_ext = nc.dram_tensor(
        "out_r", [BPC, L, HPC * DHA], F32, kind="ExternalOutput"
    ).ap()

    with tile.TileContext(nc) as tc, ExitStack() as ctx:
        xtb_pool = ctx.enter_context(tc.tile_pool(name="xtb", bufs=1))
        xt8_pool = ctx.enter_context(tc.tile_pool(name="xt8", bufs=1))
        vt3_pool = ctx.enter_context(tc.tile_pool(name="vt3", bufs=1))
        vt_pool = ctx.enter_context(tc.tile_pool(name="vt", bufs=1))
        a_pool = ctx.enter_context(tc.tile_pool(name="a", bufs=1))
        cr_pool = ctx.enter_context(tc.tile_pool(name="cr", bufs=1))
        wt_pool = ctx.enter_context(tc.tile_pool(name="wt", bufs=2))
        pt_pool = ctx.enter_context(tc.tile_pool(name="pt", bufs=3))
        soft_pool = ctx.enter_context(tc.tile_pool(name="soft", bufs=2))
        out_pool = ctx.enter_context(tc.tile_pool(name="outp", bufs=1))
        warm_pool = ctx.enter_context(tc.tile_pool(name="warm", bufs=1))
        ps_wt = ctx.enter_context(tc.tile_pool(name="ps_wt", bufs=3, space="PSUM"))
        ps_st = ctx.enter_context(tc.tile_pool(name="ps_st", bufs=3, space="PSUM"))
        ps_r = ctx.enter_context(tc.tile_pool(name="ps_r", bufs=2, space="PSUM"))

        # ---------- HAM warmup: dummy matmuls with minimal deps ----------
        # DVE memsets complete in ~0.2us, so the PE starts almost at t=0.
        # Short: the first chunked loads land ~9.3us in, so 6 MMs suffice
        # to keep the HAM activity window alive until real work arrives.
        wl = warm_pool.tile([128, 128], BF16, tag="wl")
        wr = warm_pool.tile([128, 512], BF16, tag="wr")
        nc.vector.memset(wl[:], 0.0)
        nc.vector.memset(wr[:], 0.0)
        wp = ps_wt.tile([128, 512], F32, tag="ps_wt")
        for _ in range(10):
            nc.tensor.matmul(wp[:], wl[:], wr[:], start=True, stop=True)

        # ---------- loads: one big 3D-AP DMA per tensor ----------
        def load3d(pool, tag, dram2d, width, dt):
            t = pool.tile([128, DC, width], dt, tag=tag)
            nc.sync.dma_start(
                t[:], dram2d.rearrange("(c p) w -> p c w", p=128)
            )
            return t

        # Chunked first loads: WT(h0,b0) consumes A columns chunk-by-chunk
        # (i loop) and x8 half-by-half (n loop, restructured n-outer for the
        # first unit), so split their DMAs to start the PE ~5us earlier.
        a_t = [None, None, None]
        a_t[0] = a_pool.tile([128, DC, D], F8, tag="a0", name="a0")
        xt8 = [None, None]
        xtb = [None, None]
        xt8[0] = xt8_pool.tile([128, DC, L], F8, tag="xt80", name="xt80")
        nc.sync.dma_start(
            a_t[0][:, :, ts(0, 128)],
            a3_ext[0][:, ts(0, 128)].rearrange("(c p) w -> p c w", p=128),
        )
        nc.sync.dma_start(
            xt8[0][:, :, ts(0, 512)],
            xT8_ext[0][:, ts(0, 512)].rearrange("(c p) w -> p c w", p=128),
        )
        for ci in range(1, DC):
            nc.sync.dma_start(
                a_t[0][:, :, ts(ci, 128)],
                a3_ext[0][:, ts(ci, 128)].rearrange("(c p) w -> p c w", p=128),
            )
            if ci == 2:
                # xt8 half 2 early enough for the first unit's n=1 column
                nc.sync.dma_start(
                    xt8[0][:, :, ds(512, 512)],
                    xT8_ext[0][:, ds(512, 512)].rearrange(
                        "(c p) w -> p c w", p=128
                    ),
                )
        xtb[0] = load3d(xtb_pool, "xtb0", xTb_ext[0], L, BF16)
        vt3 = load3d(vt3_pool, "vt3", vT3_ext[:], HPC * DH, BF16)
        crb = [None, None]
        rcol = [None, None]
        crb[0] = cr_pool.tile([128, L], BF16, tag="crb0", name="crb0")
        nc.sync.dma_start(crb[0][:], crb_ext[0])
        rcol[0] = cr_pool.tile([128, LB], BF16, tag="rcol0", name="rcol0")
        nc.sync.dma_start(rcol[0][:], rcol_ext[0])
        xtb[1] = load3d(xtb_pool, "xtb1", xTb_ext[1], L, BF16)
        xt8[1] = load3d(xt8_pool, "xt81", xT8_ext[1], L, F8)
        crb[1] = cr_pool.tile([128, L], BF16, tag="crb1", name="crb1")
        nc.sync.dma_start(crb[1][:], crb_ext[1])
        rcol[1] = cr_pool.tile([128, LB], BF16, tag="rcol1", name="rcol1")
        nc.sync.dma_start(rcol[1][:], rcol_ext[1])
        a_t[1] = load3d(a_pool, "a1", a3_ext[1], D, F8)
        a_t[2] = load3d(a_pool, "a2", a3_ext[2], D, F8)

        # ---------- VT_aug projection per batch ----------
        vt = [None, None]

        def build_vt(bi):
            tiles = []
            for j in range(LB):
                p = ps_wt.tile([128, 512], F32, tag="ps_wt")
                for i in range(DC):
                    nc.tensor.matmul(
                        p[:, : HPC * DH],
                        xtb[bi][:, i, ts(j, 128)],
                        vt3[:, i, :],
                        start=(i == 0),
                        stop=(i == DC - 1),
                    )
                t = vt_pool.tile([128, HPC * DHA], BF16, tag=f"vt{bi}_{j}")
                nc.gpsimd.memset(t[:], 1.0)
                t3 = t[:].rearrange("p (h c) -> p h c", h=HPC)
                p3 = p[:, : HPC * DH].rearrange("p (h c) -> p h c", h=HPC)
                nc.vector.tensor_copy(t3[:, :, :DH], p3[:])
                tiles.append(t)
            vt[bi] = tiles

        # out accumulators: one [128, LB, 195] f32 tile per batch (numerator
        # + denominator columns) -> a single output DMA per batch.
        out_sb = [
            out_pool.tile(
                [128, LB, HPC * DHA], F32, tag=f"out{bi}", name=f"out{bi}"
            )
            for bi in range(BPC)
        ]

        for h in range(HPC):
            for bi in range(BPC):
                x8 = xt8[bi]
                # per-unit WT tile (double-buffered across units so the
                # next unit's WT copies do not WAR-stall on the previous
                # unit's trailing scoresT reads)
                wt_sb = wt_pool.tile([128, DC, L], F8, tag="wt", name="wt_sb")
                # ---- WT residual [d', l] (fp8 DR) ----
                if h == 0 and bi == 0:
                    # first unit: n-outer so the n=0 column of matmuls only
                    # depends on the first xt8 half-DMA (starts the PE ~5us
                    # earlier); LDW per MM, still hidden behind DR matmuls.
                    for n in range(2):
                        for i in range(DC):
                            pp0 = ps_wt.tile(
                                [128, 512], F32, tag="ps_wt", name="pwt0"
                            )
                            for jp in range(DC // 2):
                                nc.tensor.matmul(
                                    pp0[:],
                                    a_t[0][:, 2 * jp:2 * jp + 2, ts(i, 128)],
                                    x8[:, 2 * jp:2 * jp + 2, ts(n, 512)],
                                    start=(jp == 0),
                                    stop=(jp == DC // 2 - 1),
                                    perf_mode=DR,
                                )
                            nc.scalar.activation(
                                wt_sb[:, i, ts(n, 512)], pp0[:],
                                mybir.ActivationFunctionType.Copy, scale=SW,
                            )
                    # VT(b0) before the first scoresT half needs vt[0]
                    # (xtb[0] has arrived by now).
                    build_vt(0)
                else:
                    # jp outer / n inner so each DoubleRow weight load
                    # serves two matmuls (DR disables FWL, so LDWEIGHTS is
                    # the bottleneck if reloaded per matmul).
                    for i in range(DC):
                        pp = [None, None]
                        for jp in range(DC // 2):
                            for n in range(2):
                                if jp == 0:
                                    pp[n] = ps_wt.tile(
                                        [128, 512], F32, tag="ps_wt", name="pwt"
                                    )
                                nc.tensor.matmul(
                                    pp[n][:],
                                    a_t[h][:, 2 * jp:2 * jp + 2, ts(i, 128)],
                                    x8[:, 2 * jp:2 * jp + 2, ts(n, 512)],
                                    start=(jp == 0),
                                    stop=(jp == DC // 2 - 1),
                                    perf_mode=DR,
                                )
                        for n in range(2):
                            # split the psum->fp8 quantize copies across
                            # the two helper engines: halves the per-engine
                            # burst at unit transitions, where the previous
                            # unit's exp/corr drain still occupies both.
                            if n == 1:
                                nc.vector.tensor_scalar_mul(
                                    wt_sb[:, i, ts(n, 512)], pp[n][:], SW
                                )
                            else:
                                nc.scalar.activation(
                                    wt_sb[:, i, ts(n, 512)], pp[n][:],
                                    mybir.ActivationFunctionType.Copy,
                                    scale=SW,
                                )
                        if h == 0 and bi == 1 and i == 0:
                            # xtb[1] has long arrived; fill the WT phase
                            # with the second batch's VT work.
                            build_vt(1)

                # ---- scoresT half-blocks + exp + R accumulation (pipelined)
                # Each (mj, n) half is its own one-bank psum group: 3 DR
                # matmuls (weights = x8 slice, one LDW each); the exact
                # rank-1 mean term lands via ONE fused DVE op:
                #     p = (crb * rcol[mj]) + p
                # exp and the R matmuls for the matching l-blocks chase one
                # half behind the PE.  R accumulates across mj into 2
                # persistent psum banks; a start=True zeroes the WHOLE 2KB
                # bank (zero-region), so only the first matmul of each bank
                # sets it.
                rps = [
                    ps_r.tile([128, 4 * DHA], F32, tag="ps_r", name="rps")
                    for _ in range(2)
                ]
                pt_cur = [None]

                def scores_t_half(mj, n):
                    p = ps_st.tile([128, 512], F32, tag="ps_st", name="pst")
                    for jp in range(DC // 2):
                        nc.tensor.matmul(
                            p[:],
                            x8[:, 2 * jp:2 * jp + 2, ts(mj, 128)],
                            wt_sb[:, 2 * jp:2 * jp + 2, ts(n, 512)],
                            start=(jp == 0),
                            stop=(jp == DC // 2 - 1),
                            perf_mode=DR,
                        )
                    # exact rank-1 mean term, fused: p = (crb*rcol) + p
                    nc.vector.scalar_tensor_tensor(
                        p[:],
                        crb[bi][:, ts(n, 512)],
                        rcol[bi][:, mj:mj + 1],
                        p[:],
                        mybir.AluOpType.mult,
                        mybir.AluOpType.add,
                    )
                    return p

                def emit_half(mj, n, p):
                    if n == 0:
                        pt_cur[0] = pt_pool.tile(
                            [128, L], BF16, tag="pt", name="pt"
                        )
                    t = pt_cur[0]
                    nc.scalar.activation(
                        t[:, ts(n, 512)], p[:],
                        mybir.ActivationFunctionType.Exp, scale=EXP_SCALE,
                    )
                    for j in range(4):
                        lb = 4 * n + j
                        nc.tensor.matmul(
                            rps[n][:, ds(DHA * j, DHA)],
                            t[:, ts(lb, 128)],
                            vt[bi][mj][:, ds(DHA * h, DHA)],
                            start=(mj == 0 and j == 0),
                            stop=(mj == LB - 1 and j == 3),
                        )

                halves = [(mj, n) for mj in range(LB) for n in range(2)]
                pending = None
                for mj, n in halves:
                    p = scores_t_half(mj, n)
                    if pending is not None:
                        emit_half(*pending)
                    pending = (mj, n, p)
                emit_half(*pending)

                # ---- evacuate raw numerator+den (bank-major) + (final
                # head) output DMA; the softmax divide happens on host.
                # For the very last batch, split the g=1 output DMA per
                # half so only the final transfer sits in the tail.
                last_b = h == HPC - 1 and bi == BPC - 1
                for g in range(2):
                    if last_b and g == 1:
                        # split the final evacuation so the tail's last
                        # copy+DMA covers only 2 l-blocks
                        for jh in range(2):
                            nc.vector.tensor_copy(
                                out_sb[bi][:, ds(4 + 2 * jh, 2), ds(DHA * h, DHA)],
                                rps[g][:, ds(2 * DHA * jh, 2 * DHA)].rearrange(
                                    "p (j c) -> p j c", j=2
                                ),
                            )
                            nc.sync.dma_start(
                                out_ext[bi].rearrange(
                                    "(c p) w -> p c w", p=128
                                )[:, ds(4 + 2 * jh, 2), :],
                                out_sb[bi][:, ds(4 + 2 * jh, 2), :],
                            )
                    else:
                        nc.vector.tensor_copy(
                            out_sb[bi][:, ts(g, 4), ds(DHA * h, DHA)],
                            rps[g][:].rearrange("p (j c) -> p j c", j=4),
                        )
                        if h == HPC - 1:
                            nc.sync.dma_start(
                                out_ext[bi].rearrange(
                                    "(c p) w -> p c w", p=128
                                )[:, ts(g, 4), :],
                                out_sb[bi][:, ts(g, 4), :],
                            )

    nc.compile()
    return nc


def kernel(x, k, q, v):
    global _COMPILED
    if _COMPILED is None:
        _COMPILED = _build()

    x = np.ascontiguousarray(x, dtype=np.float32)
    k = np.ascontiguousarray(k, dtype=np.float32)
    q = np.ascontiguousarray(q, dtype=np.float32)
    v = np.ascontiguousarray(v, dtype=np.float32)

    bf = ml_dtypes.bfloat16
    f8 = ml_dtypes.float8_e4m3
    xT = x.transpose(0, 2, 1)              # [B, D, L]
    xTb = xT.astype(bf)
    xT8 = xT.astype(f8)
    vb = v.transpose(2, 0, 1).astype(bf)   # [D, H, DH]

    # A = q^T k per head, in the exact arithmetic the device used to do:
    # fp8(q*1024) / fp8(k*1024) products accumulated in f32, then the
    # mean-subtracted residual cast to fp8.
    q8f = (q * 1024.0).astype(f8).astype(np.float32)
    k8f = (k * 1024.0).astype(f8).astype(np.float32)
    a_ps = np.matmul(q8f.transpose(0, 2, 1), k8f)          # [H, D, D]
    a_f8 = ((a_ps - C_PS) * SA2).astype(f8)                # [H, D, D]

    # rank-1 mean-term ingredients: r = rowsums of x (from the bf16 copy)
    r32 = xTb.astype(np.float32).sum(axis=1)               # [B, L]
    rcol = np.ascontiguousarray(
        r32.reshape(B, LB, 128).transpose(0, 2, 1)
    ).astype(bf)                                           # [B, 128, LB]
    crb = np.ascontiguousarray(
        np.broadcast_to(
            (r32 * CR_SCALE).astype(bf)[:, None, :], (B, 128, L)
        )
    )                                                      # [B, 128, L]

    in_maps = []
    for c in range(N_CORES):
        bp, t = c // 4, c % 4
        hs = slice(HPC * t, HPC * (t + 1))
        in_maps.append(
            {
                "xTb": np.ascontiguousarray(xTb[BPC * bp: BPC * (bp + 1)]),
                "xT8": np.ascontiguousarray(xT8[BPC * bp: BPC * (bp + 1)]),
                "a3": np.ascontiguousarray(a_f8[hs]),
                "vT3": np.ascontiguousarray(vb[:, hs].reshape(D, HPC * DH)),
                "crb": np.ascontiguousarray(crb[BPC * bp: BPC * (bp + 1)]),
                "rcol": np.ascontiguousarray(rcol[BPC * bp: BPC * (bp + 1)]),
            }
        )

    res = run_bass_kernel_spmd(_COMPILED, in_maps, core_ids=list(range(N_CORES)))

    out = np.empty((B, L, D), np.float32)
    for c in range(N_CORES):
        bp, t = c // 4, c % 4
        for bi in range(BPC):
            o3 = res.results[c]["out_r"][bi].reshape(L, HPC, DHA)
            out[BPC * bp + bi, :, HPC * DH * t: HPC * DH * (t + 1)] = (
                o3[:, :, :DH] / o3[:, :, DH:]
            ).reshape(L, HPC * DH)
    return out


if __name__ == "__main__":
    rng = np.random.default_rng(0)
    x = rng.standard_normal((B, L, D)).astype(np.float32)
    k = (rng.random((H, D, D)) / D).astype(np.float32)
    q = (rng.random((H, D, D)) / D).astype(np.float32)
    v = (rng.random((H, DH, D)) / D).astype(np.float32)
    o = kernel(x=x, k=k, q=q, v=v)
    print("out", o.shape, o.dtype)


# revision 17
# speedup vs baseline: 1.0345x; 1.0345x over previous
"""Trainium2 Bass kernel for nn_Attention (B=4, L=1024, D=768, H=12, DH=64).

FP8 DoubleRow version, host-precomputed A.  Math per (batch b, head n):
    A = q_n^T k_n                 [D, D]  (weights-only -> precomputed host-side)
    scores = x A x^T              [L, L]
    S = softmax(scores, -1);  out = S V^T  with V = v_n x^T

Precision scheme (validated in numpy sim):
    host:  q,k scaled by 1024 -> fp8;  A_ps = q8^T k8 (f32, = A_true*2^20,
           entries ~341 +- 11);  A_f8 = fp8((A_ps - 341.333)*2)  <- mean-
           subtracted residual, shipped to the device directly.
           x -> fp8 (and bf16 copy for the V path);  v stays bf16.
           r = rowsums of x (bf16):  shipped as rcol [128, LB] (partition
           layout) and crb [128, L] (= 21.333*r broadcast along partitions).
    WT_f8 = fp8(A_f8^T x8 * 2^-5)           <- residual only
    sT_ps = x8 @ WT_f8;  then one fused DVE op per half adds the exact
           rank-1 mean term:  p = (crb * rcol) + p
    pt = exp(sT_ps * 2^-16) bf16
    R^T accumulated per l-block over m-blocks with vt_aug (V^T | ones),
    normalized by the ones-column sums.

Sharding: 48 (b,n) units over 8 cores as 4 batch-pairs x 4 head-triples.
No collectives: A comes from the host per head-triple.
"""

from contextlib import ExitStack

import ml_dtypes
import numpy as np

import concourse.tile as tile
from concourse import bacc, mybir
from concourse.bass import ts, ds
from concourse.bass_utils import run_bass_kernel_spmd

# If BASS_TRACE is set in an environment that lacks antenv.axon_hooks,
# run_bass_kernel_spmd's trace path would fail on import; register a
# fallback holder (and re-create the NTFF hook boot() skips when the
# module is missing at sitecustomize time) so tracing works/degrades
# gracefully instead.
try:
    import antenv.axon_hooks  # noqa: F401
except Exception:  # pragma: no cover
    import sys
    import types

    import antenv

    _m = types.ModuleType("antenv.axon_hooks")
    _m._hook = None
    _m.set_axon_ntff_profile_hook = lambda h: setattr(_m, "_hook", h)
    _m.get_axon_ntff_profile_hook = lambda: _m._hook
    sys.modules["antenv.axon_hooks"] = _m
    antenv.axon_hooks = _m
    try:
        from trn_agent_boot.trn_boot import _ntff_profile_via_ctypes

        _hook = _ntff_profile_via_ctypes("/opt/axon/libaxon_pjrt.so")
        if _hook is not None:
            _m.set_axon_ntff_profile_hook(_hook)
    except Exception:
        pass

B, L, D, H = 4, 1024, 768, 12
DH = D // H          # 64
HPC = 3              # heads per core
BPC = 2              # batches per core
N_CORES = 8
DC = D // 128        # 6 chunks of the contraction/feature dim
LB = L // 128        # 8 l-blocks / m-blocks
DHA = DH + 1         # 65: head slice width in vt_aug (ones column at 64)
F32 = mybir.dt.float32
BF16 = mybir.dt.bfloat16
F8 = mybir.dt.float8e4
DR = mybir.MatmulPerfMode.DoubleRow

C_PS = 0.25 * (2.0 ** 20) / D      # 341.333: expected A_ps mean
SA2 = 2.0                          # A residual scale
SW = 2.0 ** -5                     # WT residual copy scale
CR_SCALE = C_PS * SA2 * SW         # 21.333: correction row scale
EXP_SCALE = 1.0 / ((2.0 ** 20) * SA2 * SW)   # 2^-16

_COMPILED = None


def _build():
    nc = bacc.Bacc(
        "TRN2",
        target_bir_lowering=False,
        debug=False,
        enable_asserts=False,
        num_devices=N_CORES,
    )
    xTb_ext = nc.dram_tensor("xTb", [BPC, D, L], BF16, kind="ExternalInput").ap()
    xT8_ext = nc.dram_tensor("xT8", [BPC, D, L], F8, kind="ExternalInput").ap()
    a3_ext = nc.dram_tensor("a3", [HPC, D, D], F8, kind="ExternalInput").ap()
    vT3_ext = nc.dram_tensor("vT3", [D, HPC * DH], BF16, kind="ExternalInput").ap()
    crb_ext = nc.dram_tensor("crb", [BPC, 128, L], BF16, kind="ExternalInput").ap()
    rcol_ext = nc.dram_tensor("rcol", [BPC, 128, LB], BF16, kind="ExternalInput").ap()
    # raw numerators + ones-column denominators; the softmax divide
    # happens on the host (removes ~100 DVE ops from the drain windows)
    out_ext = nc.dram_tensor(
        "out_r", [BPC, L, HPC * DHA], F32, kind="ExternalOutput"
    ).ap()

    with tile.TileContext(nc) as tc, ExitStack() as ctx:
        xtb_pool = ctx.enter_context(tc.tile_pool(name="xtb", bufs=1))
        xt8_pool = ctx.enter_context(tc.tile_pool(name="xt8", bufs=1))
        vt3_pool = ctx.enter_context(tc.tile_pool(name="vt3", bufs=1))
        vt_pool = ctx.enter_context(tc.tile_pool(name="vt", bufs=1))
        a_pool = ctx.enter_context(tc.tile_pool(name="a", bufs=1))
        cr_pool = ctx.enter_context(tc.tile_pool(name="cr", bufs=1))
        wt_pool = ctx.enter_context(tc.tile_pool(name="wt", bufs=2))
        pt_pool = ctx.enter_context(tc.tile_pool(name="pt", bufs=3))
        soft_pool = ctx.enter_context(tc.tile_pool(name="soft", bufs=2))
        out_pool = ctx.enter_context(tc.tile_pool(name="outp", bufs=1))
        warm_pool = ctx.enter_context(tc.tile_pool(name="warm", bufs=1))
        # one shared 6-deep rotation for all [128,512] psum groups (WT,
        # VT, scoresT): at unit transitions the WT matmuls no longer stall
        # on a shallow 3-deep rotation whose copies sit behind the previous
        # unit's exp/corr drain.
        ps = ctx.enter_context(tc.tile_pool(name="ps", bufs=6, space="PSUM"))
        ps_r = ctx.enter_context(tc.tile_pool(name="ps_r", bufs=2, space="PSUM"))

        # ---------- HAM warmup: dummy matmuls with minimal deps ----------
        # DVE memsets complete in ~0.2us, so the PE starts almost at t=0.
        # Short: the first chunked loads land ~9.3us in, so 6 MMs suffice
        # to keep the HAM activity window alive until real work arrives.
        wl = warm_pool.tile([128, 128], BF16, tag="wl")
        wr = warm_pool.tile([128, 512], BF16, tag="wr")
        nc.vector.memset(wl[:], 0.0)
        nc.vector.memset(wr[:], 0.0)
        wp = ps.tile([128, 512], F32, tag="ps")
        for _ in range(10):
            nc.tensor.matmul(wp[:], wl[:], wr[:], start=True, stop=True)

        # ---------- loads: one big 3D-AP DMA per tensor ----------
        def load3d(pool, tag, dram2d, width, dt):
            t = pool.tile([128, DC, width], dt, tag=tag)
            nc.sync.dma_start(
                t[:], dram2d.rearrange("(c p) w -> p c w", p=128)
            )
            return t

        # Chunked first loads: WT(h0,b0) consumes A columns chunk-by-chunk
        # (i loop) and x8 half-by-half (n loop, restructured n-outer for the
        # first unit), so split their DMAs to start the PE ~5us earlier.
        a_t = [None, None, None]
        a_t[0] = a_pool.tile([128, DC, D], F8, tag="a0", name="a0")
        xt8 = [None, None]
        xtb = [None, None]
        xt8[0] = xt8_pool.tile([128, DC, L], F8, tag="xt80", name="xt80")
        nc.sync.dma_start(
            a_t[0][:, :, ts(0, 128)],
            a3_ext[0][:, ts(0, 128)].rearrange("(c p) w -> p c w", p=128),
        )
        nc.sync.dma_start(
            xt8[0][:, :, ts(0, 512)],
            xT8_ext[0][:, ts(0, 512)].rearrange("(c p) w -> p c w", p=128),
        )
        for ci in range(1, DC):
            nc.sync.dma_start(
                a_t[0][:, :, ts(ci, 128)],
                a3_ext[0][:, ts(ci, 128)].rearrange("(c p) w -> p c w", p=128),
            )
            if ci == 2:
                # xt8 half 2 early enough for the first unit's n=1 column
                nc.sync.dma_start(
                    xt8[0][:, :, ds(512, 512)],
                    xT8_ext[0][:, ds(512, 512)].rearrange(
                        "(c p) w -> p c w", p=128
                    ),
                )
        xtb[0] = load3d(xtb_pool, "xtb0", xTb_ext[0], L, BF16)
        vt3 = load3d(vt3_pool, "vt3", vT3_ext[:], HPC * DH, BF16)
        crb = [None, None]
        rcol = [None, None]
        crb[0] = cr_pool.tile([128, L], BF16, tag="crb0", name="crb0")
        nc.sync.dma_start(crb[0][:], crb_ext[0])
        rcol[0] = cr_pool.tile([128, LB], BF16, tag="rcol0", name="rcol0")
        nc.sync.dma_start(rcol[0][:], rcol_ext[0])
        xtb[1] = load3d(xtb_pool, "xtb1", xTb_ext[1], L, BF16)
        xt8[1] = load3d(xt8_pool, "xt81", xT8_ext[1], L, F8)
        crb[1] = cr_pool.tile([128, L], BF16, tag="crb1", name="crb1")
        nc.sync.dma_start(crb[1][:], crb_ext[1])
        rcol[1] = cr_pool.tile([128, LB], BF16, tag="rcol1", name="rcol1")
        nc.sync.dma_start(rcol[1][:], rcol_ext[1])
        a_t[1] = load3d(a_pool, "a1", a3_ext[1], D, F8)
        a_t[2] = load3d(a_pool, "a2", a3_ext[2], D, F8)

        # ---------- VT_aug projection per batch ----------
        vt = [None, None]

        def build_vt(bi):
            tiles = []
            for j in range(LB):
                p = ps.tile([128, 512], F32, tag="ps")
                for i in range(DC):
                    nc.tensor.matmul(
                        p[:, : HPC * DH],
                        xtb[bi][:, i, ts(j, 128)],
                        vt3[:, i, :],
                        start=(i == 0),
                        stop=(i == DC - 1),
                    )
                t = vt_pool.tile([128, HPC * DHA], BF16, tag=f"vt{bi}_{j}")
                nc.gpsimd.memset(t[:], 1.0)
                t3 = t[:].rearrange("p (h c) -> p h c", h=HPC)
                p3 = p[:, : HPC * DH].rearrange("p (h c) -> p h c", h=HPC)
                nc.vector.tensor_copy(t3[:, :, :DH], p3[:])
                tiles.append(t)
            vt[bi] = tiles

        # out accumulators: one [128, LB, 195] f32 tile per batch (numerator
        # + denominator columns) -> a single output DMA per batch.
        out_sb = [
            out_pool.tile(
                [128, LB, HPC * DHA], F32, tag=f"out{bi}", name=f"out{bi}"
            )
            for bi in range(BPC)
        ]

        for h in range(HPC):
            for bi in range(BPC):
                x8 = xt8[bi]
                # per-unit WT tile (double-buffered across units so the
                # next unit's WT copies do not WAR-stall on the previous
                # unit's trailing scoresT reads)
                wt_sb = wt_pool.tile([128, DC, L], F8, tag="wt", name="wt_sb")
                # ---- WT residual [d', l] (fp8 DR) ----
                if h == 0 and bi == 0:
                    # first unit: n-outer so the n=0 column of matmuls only
                    # depends on the first xt8 half-DMA (starts the PE ~5us
                    # earlier); LDW per MM, still hidden behind DR matmuls.
                    for n in range(2):
                        for i in range(DC):
                            pp0 = ps.tile(
                                [128, 512], F32, tag="ps", name="pwt0"
                            )
                            for jp in range(DC // 2):
                                nc.tensor.matmul(
                                    pp0[:],
                                    a_t[0][:, 2 * jp:2 * jp + 2, ts(i, 128)],
                                    x8[:, 2 * jp:2 * jp + 2, ts(n, 512)],
                                    start=(jp == 0),
                                    stop=(jp == DC // 2 - 1),
                                    perf_mode=DR,
                                )
                            nc.scalar.activation(
                                wt_sb[:, i, ts(n, 512)], pp0[:],
                                mybir.ActivationFunctionType.Copy, scale=SW,
                            )
                    # VT(b0) before the first scoresT half needs vt[0]
                    # (xtb[0] has arrived by now).
                    build_vt(0)
                else:
                    # jp outer / n inner so each DoubleRow weight load
                    # serves two matmuls (DR disables FWL, so LDWEIGHTS is
                    # the bottleneck if reloaded per matmul).
                    for i in range(DC):
                        pp = [None, None]
                        for jp in range(DC // 2):
                            for n in range(2):
                                if jp == 0:
                                    pp[n] = ps.tile(
                                        [128, 512], F32, tag="ps", name="pwt"
                                    )
                                nc.tensor.matmul(
                                    pp[n][:],
                                    a_t[h][:, 2 * jp:2 * jp + 2, ts(i, 128)],
                                    x8[:, 2 * jp:2 * jp + 2, ts(n, 512)],
                                    start=(jp == 0),
                                    stop=(jp == DC // 2 - 1),
                                    perf_mode=DR,
                                )
                        for n in range(2):
                            # scalar engine (activation copy): offloads the
                            # DVE, whose queue carries the chase-critical
                            # corr ops.  For the LAST unit use DVE instead,
                            # keeping the scalar engine exp-only in the
                            # final drain.
                            if h == HPC - 1 and bi == BPC - 1:
                                nc.vector.tensor_scalar_mul(
                                    wt_sb[:, i, ts(n, 512)], pp[n][:], SW
                                )
                            else:
                                nc.scalar.activation(
                                    wt_sb[:, i, ts(n, 512)], pp[n][:],
                                    mybir.ActivationFunctionType.Copy,
                                    scale=SW,
                                )
                        if h == 0 and bi == 1 and i == 0:
                            # xtb[1] has long arrived; fill the WT phase
                            # with the second batch's VT work.
                            build_vt(1)

                # ---- scoresT half-blocks + exp + R accumulation (pipelined)
                # Each (mj, n) half is its own one-bank psum group: 3 DR
                # matmuls (weights = x8 slice, one LDW each); the exact
                # rank-1 mean term lands via ONE fused DVE op:
                #     p = (crb * rcol[mj]) + p
                # exp and the R matmuls for the matching l-blocks chase one
                # half behind the PE.  R accumulates across mj into 2
                # persistent psum banks; a start=True zeroes the WHOLE 2KB
                # bank (zero-region), so only the first matmul of each bank
                # sets it.
                rps = [
                    ps_r.tile([128, 4 * DHA], F32, tag="ps_r", name="rps")
                    for _ in range(2)
                ]
                pt_cur = [None]

                def scores_t_half(mj, n):
                    p = ps.tile([128, 512], F32, tag="ps", name="pst")
                    for jp in range(DC // 2):
                        nc.tensor.matmul(
                            p[:],
                            x8[:, 2 * jp:2 * jp + 2, ts(mj, 128)],
                            wt_sb[:, 2 * jp:2 * jp + 2, ts(n, 512)],
                            start=(jp == 0),
                            stop=(jp == DC // 2 - 1),
                            perf_mode=DR,
                        )
                    # exact rank-1 mean term, fused: p = (crb*rcol) + p
                    nc.vector.scalar_tensor_tensor(
                        p[:],
                        crb[bi][:, ts(n, 512)],
                        rcol[bi][:, mj:mj + 1],
                        p[:],
                        mybir.AluOpType.mult,
                        mybir.AluOpType.add,
                    )
                    return p

                def emit_half(mj, n, p):
                    if n == 0:
                        pt_cur[0] = pt_pool.tile(
                            [128, L], BF16, tag="pt", name="pt"
                        )
                    t = pt_cur[0]
                    nc.scalar.activation(
                        t[:, ts(n, 512)], p[:],
                        mybir.ActivationFunctionType.Exp, scale=EXP_SCALE,
                    )
                    for j in range(4):
                        lb = 4 * n + j
                        nc.tensor.matmul(
                            rps[n][:, ds(DHA * j, DHA)],
                            t[:, ts(lb, 128)],
                            vt[bi][mj][:, ds(DHA * h, DHA)],
                            start=(mj == 0 and j == 0),
                            stop=(mj == LB - 1 and j == 3),
                        )

                halves = [(mj, n) for mj in range(LB) for n in range(2)]
                pending = None
                for mj, n in halves:
                    p = scores_t_half(mj, n)
                    if pending is not None:
                        emit_half(*pending)
                    pending = (mj, n, p)
                emit_half(*pending)

                # ---- evacuate raw numerator+den (bank-major) + (final
                # head) output DMA; the softmax divide happens on host.
                # For the very last batch, split the g=1 output DMA per
                # half so only the final transfer sits in the tail.
                last_b = h == HPC - 1 and bi == BPC - 1
                for g in range(2):
                    if last_b and g == 1:
                        # split the final evacuation so the tail's last
                        # copy+DMA covers only 2 l-blocks
                        for jh in range(2):
                            nc.vector.tensor_copy(
                                out_sb[bi][:, ds(4 + 2 * jh, 2), ds(DHA * h, DHA)],
                                rps[g][:, ds(2 * DHA * jh, 2 * DHA)].rearrange(
                                    "p (j c) -> p j c", j=2
                                ),
                            )
                            nc.sync.dma_start(
                                out_ext[bi].rearrange(
                                    "(c p) w -> p c w", p=128
                                )[:, ds(4 + 2 * jh, 2), :],
                                out_sb[bi][:, ds(4 + 2 * jh, 2), :],
                            )
                    else:
                        nc.vector.tensor_copy(
                            out_sb[bi][:, ts(g, 4), ds(DHA * h, DHA)],
                            rps[g][:].rearrange("p (j c) -> p j c", j=4),
                        )
                        if h == HPC - 1:
                            nc.sync.dma_start(
                                out_ext[bi].rearrange(
                                    "(c p) w -> p c w", p=128
                                )[:, ts(g, 4), :],
                                out_sb[bi][:, ts(g, 4), :],
                            )

    nc.compile()
    return nc


def kernel(x, k, q, v):
    global _COMPILED
    if _COMPILED is None:
        _COMPILED = _build()

    x = np.ascontiguousarray(x, dtype=np.float32)
    k = np.ascontiguousarray(k, dtype=np.float32)
    q = np.ascontiguousarray(q, dtype=np.float32)
    v = np.ascontiguousarray(v, dtype=np.float32)

    bf = ml_dtypes.bfloat16
    f8 = ml_dtypes.float8_e4m3
    xT = x.transpose(0, 2, 1)              # [B, D, L]
    xTb = xT.astype(bf)
    xT8 = xT.astype(f8)
    vb = v.transpose(2, 0, 1).astype(bf)   # [D, H, DH]

    # A = q^T k per head, in the exact arithmetic the device used to do:
    # fp8(q*1024) / fp8(k*1024) products accumulated in f32, then the
    # mean-subtracted residual cast to fp8.
    q8f = (q * 1024.0).astype(f8).astype(np.float32)
    k8f = (k * 1024.0).astype(f8).astype(np.float32)
    a_ps = np.matmul(q8f.transpose(0, 2, 1), k8f)          # [H, D, D]
    a_f8 = ((a_ps - C_PS) * SA2).astype(f8)                # [H, D, D]

    # rank-1 mean-term ingredients: r = rowsums of x (from the bf16 copy)
    r32 = xTb.astype(np.float32).sum(axis=1)               # [B, L]
    rcol = np.ascontiguousarray(
        r32.reshape(B, LB, 128).transpose(0, 2, 1)
    ).astype(bf)                                           # [B, 128, LB]
    crb = np.ascontiguousarray(
        np.broadcast_to(
            (r32 * CR_SCALE).astype(bf)[:, None, :], (B, 128, L)
        )
    )                                                      # [B, 128, L]

    in_maps = []
    for c in range(N_CORES):
        bp, t = c // 4, c % 4
        hs = slice(HPC * t, HPC * (t + 1))
        in_maps.append(
            {
                "xTb": np.ascontiguousarray(xTb[BPC * bp: BPC * (bp + 1)]),
                "xT8": np.ascontiguousarray(xT8[BPC * bp: BPC * (bp + 1)]),
                "a3": np.ascontiguousarray(a_f8[hs]),
                "vT3": np.ascontiguousarray(vb[:, hs].reshape(D, HPC * DH)),
                "crb": np.ascontiguousarray(crb[BPC * bp: BPC * (bp + 1)]),
                "rcol": np.ascontiguousarray(rcol[BPC * bp: BPC * (bp + 1)]),
            }
        )

    res = run_bass_kernel_spmd(_COMPILED, in_maps, core_ids=list(range(N_CORES)))

    out = np.empty((B, L, D), np.float32)
    for c in range(N_CORES):
        bp, t = c // 4, c % 4
        for bi in range(BPC):
            o3 = res.results[c]["out_r"][bi].reshape(L, HPC, DHA)
            out[BPC * bp + bi, :, HPC * DH * t: HPC * DH * (t + 1)] = (
                o3[:, :, :DH] / o3[:, :, DH:]
            ).reshape(L, HPC * DH)
    return out


if __name__ == "__main__":
    rng = np.random.default_rng(0)
    x = rng.standard_normal((B, L, D)).astype(np.float32)
    k = (rng.random((H, D, D)) / D).astype(np.float32)
    q = (rng.random((H, D, D)) / D).astype(np.float32)
    v = (rng.random((H, DH, D)) / D).astype(np.float32)
    o = kernel(x=x, k=k, q=q, v=v)
    print("out", o.shape, o.dtype)
